# revision 3
# baseline (speedup 1.0000x reference)
"""Trainium2 Bass kernel: 3x3 VALID conv (NHWC, 256->256 ch) with weight
thresholding + bias, batch-sharded across 8 NeuronCores (4 images/core).

Device strategy per core:
  - x pre-transposed on host to [cin, H*W] (2 partition tiles of 128),
    loaded per image in 4 row-aligned chunks (16 out-rows each) so compute
    starts early and chunks double-buffer.
  - conv = 9 shifted matmuls per output tile accumulated in PSUM over
    9 taps x 2 cin tiles, fp32r (1 cyc/row, TF32-class precision).
  - moving operand is a 3D AP [128, rows, 62] with row stride 64: only the
    62 valid output columns per row are computed (packed output, no
    garbage columns, no padding needed).
  - bias fused into the PSUM->SBUF drain (DVE tensor_scalar_add).
"""

import sys

sys.path.insert(0, "/opt/trn_rl_repo")

import numpy as np

import concourse.bacc as bacc
import concourse.mybir as mybir
import concourse.tile as tile
from concourse.bass_utils import run_bass_kernel_spmd

F32 = mybir.dt.float32
F32R = mybir.dt.float32r

N_CORES = 8
IMG_PER_CORE = 4
C = 256
NPIX = 4096               # 64*64 input pixels per image
NV = 62 * 62              # 3844 valid output pixels per image
# 4 input-row chunks per image: (first_input_row, n_input_rows)
CHUNKS = [(0, 18), (16, 18), (32, 18), (48, 16)]
# output blocks: (out_row0, n_out_rows, chunk_idx)
BLOCKS = [(8 * b, 8 if b < 7 else 6, b // 2) for b in range(8)]
SPARSE_TH = 0.01
TAPS = [(kh, kw) for kh in range(3) for kw in range(3)]

_CACHE = {}


def _build(reps: int = 1, use_for_i: bool = False, internal_io: bool = False):
    key = (reps, use_for_i, internal_io)
    if key in _CACHE:
        return _CACHE[key]

    nc = bacc.Bacc("TRN2", target_bir_lowering=False, debug=False,
                   num_devices=N_CORES)

    io_kind = "Internal" if internal_io else None
    x_d = nc.dram_tensor("xt", [IMG_PER_CORE, 2, 128, NPIX], F32R,
                         kind=io_kind or "ExternalInput")
    w_d = nc.dram_tensor("wt", [2, 128, 9 * C], F32R,
                         kind=io_kind or "ExternalInput")
    b_d = nc.dram_tensor("bias", [128, 2], F32, kind=io_kind or "ExternalInput")
    o_d = nc.dram_tensor("out", [IMG_PER_CORE, 2, 128, NV], F32,
                         kind=io_kind or "ExternalOutput")
    if internal_io:
        di_d = nc.dram_tensor("dummy_in", [1, 4], F32, kind="ExternalInput")
        do_d = nc.dram_tensor("dummy_out", [1, 4], F32, kind="ExternalOutput")

    with tile.TileContext(nc) as tc:
        with tc.tile_pool(name="wp", bufs=1) as wp, \
             tc.tile_pool(name="xp", bufs=2) as xp, \
             tc.tile_pool(name="pp", bufs=8, space="PSUM") as pp, \
             tc.tile_pool(name="op", bufs=6) as op:

            if internal_io:
                dt_sb = wp.tile([1, 4], F32, tag="dummy")
                nc.sync.dma_start(dt_sb[:], di_d[:])
                nc.sync.dma_start(do_d[:], dt_sb[:])

            w_sb = []
            for ct in range(2):
                wt = wp.tile([128, 9 * C], F32R, tag=f"w{ct}")
                nc.sync.dma_start(wt[:], w_d[ct])
                w_sb.append(wt)
            b_sb = wp.tile([128, 2], F32, tag="bias")
            nc.sync.dma_start(b_sb[:], b_d[:])

            def body():
                for img in range(IMG_PER_CORE):
                    x_sb = [[None] * 4 for _ in range(2)]
                    for ci, (r0, nr) in enumerate(CHUNKS):
                        for ct in range(2):
                            xt = xp.tile([128, nr, 64], F32R,
                                         tag=f"x{ct}c{ci}")
                            nc.sync.dma_start(
                                xt[:], x_d[img, ct, :, r0 * 64:(r0 + nr) * 64])
                            x_sb[ct][ci] = xt
                    for y0, nrow, ci in BLOCKS:
                        n = 62 * nrow
                        p0 = 62 * y0
                        lr = y0 - CHUNKS[ci][0]
                        for co in range(2):
                            ps = pp.tile([128, n], F32, tag="ps")
                            for ct in range(2):
                                for t, (kh, kw) in enumerate(TAPS):
                                    nc.tensor.matmul(
                                        ps[:],
                                        w_sb[ct][:, t * C + co * 128:
                                                 t * C + co * 128 + 128],
                                        x_sb[ct][ci][:, lr + kh:lr + kh + nrow,
                                                     kw:kw + 62],
                                        start=(ct == 0 and t == 0),
                                        stop=(ct == 1 and t == 8),
                                    )
                            ob = op.tile([128, n], F32, tag="ob")
                            nc.vector.tensor_scalar_add(
                                ob[:], ps[:], b_sb[:, co:co + 1])
                            nc.sync.dma_start(o_d[img, co, :, p0:p0 + n],
                                              ob[:])

            if use_for_i:
                with tc.For_i(0, reps):
                    body()
            else:
                for _ in range(reps):
                    body()

    nc.compile()
    _CACHE[key] = nc
    return nc


def _prep_inputs(x, weight, bias):
    """Host-side shard prep: threshold mask + relayout. Per-core in_maps."""
    w = np.where(np.abs(weight) < SPARSE_TH, 0.0, weight).astype(np.float32)
    # (cout, cin, kh, kw) -> (cin, kh, kw, cout) -> [2, 128, 9*256]
    wt = np.ascontiguousarray(w.transpose(1, 2, 3, 0)).reshape(2, 128, 9 * C)
    b2 = np.ascontiguousarray(bias.astype(np.float32).reshape(2, 128).T)

    n_img = x.shape[0]
    xs = np.ascontiguousarray(
        x.astype(np.float32).reshape(n_img, NPIX, C).transpose(0, 2, 1))
    xs = xs.reshape(n_img, 2, 128, NPIX)

    in_maps = []
    for c in range(N_CORES):
        in_maps.append({
            "xt": np.ascontiguousarray(
                xs[c * IMG_PER_CORE:(c + 1) * IMG_PER_CORE]),
            "wt": wt,
            "bias": b2,
        })
    return in_maps


def _assemble(results):
    outs = np.concatenate([r["out"] for r in results], axis=0)  # (32,2,128,3844)
    outs = outs.reshape(32, C, 62, 62).transpose(0, 2, 3, 1)
    return np.ascontiguousarray(outs)


def kernel(x, weight, bias):
    x = np.asarray(x)
    weight = np.asarray(weight)
    bias = np.asarray(bias)
    nc = _build(reps=1)
    in_maps = _prep_inputs(x, weight, bias)
    res = run_bass_kernel_spmd(nc, in_maps, list(range(N_CORES)))
    return _assemble(res.results)



# revision 4
# speedup vs baseline: 1.0019x; 1.0019x over previous
"""Trainium2 Bass kernel: 3x3 VALID conv (NHWC, 256->256 ch) with weight
thresholding + bias, batch-sharded across 8 NeuronCores (4 images/core).

Device strategy per core:
  - x pre-transposed on host to [cin, H, W] bf16 (2 partition tiles of 128),
    one full-image tile per cin half, double-buffered across images.
  - conv = 9 shifted matmuls per output tile accumulated in PSUM over
    9 taps x 2 cin tiles, bf16 (1 cyc/row, FWL fast weight loads).
  - moving operand is a 3D AP [128, rows, 62] with row stride 64: only the
    62 valid output columns per row are computed (packed output, no
    garbage columns, no padding needed).
  - bias fused into the PSUM->SBUF drain (DVE tensor_scalar_add), fp32 out.
"""

import sys

sys.path.insert(0, "/opt/trn_rl_repo")

import ml_dtypes
import numpy as np

import concourse.bacc as bacc
import concourse.mybir as mybir
import concourse.tile as tile
from concourse.bass_utils import run_bass_kernel_spmd

F32 = mybir.dt.float32
BF16 = mybir.dt.bfloat16
BF_NP = ml_dtypes.bfloat16

N_CORES = 8
IMG_PER_CORE = 4
C = 256
NV = 62 * 62              # 3844 valid output pixels per image
# output blocks: (out_row0, n_out_rows)
BLOCKS = [(8 * b, 8 if b < 7 else 6) for b in range(8)]
SPARSE_TH = 0.01
TAPS = [(kh, kw) for kh in range(3) for kw in range(3)]

_CACHE = {}


def _build(reps: int = 1, use_for_i: bool = False, internal_io: bool = False):
    key = (reps, use_for_i, internal_io)
    if key in _CACHE:
        return _CACHE[key]

    nc = bacc.Bacc("TRN2", target_bir_lowering=False, debug=False,
                   num_devices=N_CORES)

    io_kind = "Internal" if internal_io else None
    x_d = nc.dram_tensor("xt", [IMG_PER_CORE, 2, 128, 64, 64], BF16,
                         kind=io_kind or "ExternalInput")
    w_d = nc.dram_tensor("wt", [2, 128, 9 * C], BF16,
                         kind=io_kind or "ExternalInput")
    b_d = nc.dram_tensor("bias", [128, 2], F32, kind=io_kind or "ExternalInput")
    o_d = nc.dram_tensor("out", [IMG_PER_CORE, 2, 128, NV], F32,
                         kind=io_kind or "ExternalOutput")
    if internal_io:
        di_d = nc.dram_tensor("dummy_in", [1, 4], F32, kind="ExternalInput")
        do_d = nc.dram_tensor("dummy_out", [1, 4], F32, kind="ExternalOutput")

    with tile.TileContext(nc) as tc:
        with tc.tile_pool(name="wp", bufs=1) as wp, \
             tc.tile_pool(name="xp", bufs=2) as xp, \
             tc.tile_pool(name="pp", bufs=8, space="PSUM") as pp, \
             tc.tile_pool(name="op", bufs=6) as op:

            if internal_io:
                dt_sb = wp.tile([1, 4], F32, tag="dummy")
                nc.sync.dma_start(dt_sb[:], di_d[:])
                nc.sync.dma_start(do_d[:], dt_sb[:])

            w_sb = []
            for ct in range(2):
                wt = wp.tile([128, 9 * C], BF16, tag=f"w{ct}")
                nc.sync.dma_start(wt[:], w_d[ct])
                w_sb.append(wt)
            b_sb = wp.tile([128, 2], F32, tag="bias")
            nc.sync.dma_start(b_sb[:], b_d[:])

            def body():
                for img in range(IMG_PER_CORE):
                    x_sb = []
                    for ct in range(2):
                        xt = xp.tile([128, 64, 64], BF16, tag=f"x{ct}")
                        nc.sync.dma_start(xt[:], x_d[img, ct])
                        x_sb.append(xt)
                    for y0, nrow in BLOCKS:
                        n = 62 * nrow
                        p0 = 62 * y0
                        for co in range(2):
                            ps = pp.tile([128, n], F32, tag="ps")
                            for ct in range(2):
                                for t, (kh, kw) in enumerate(TAPS):
                                    nc.tensor.matmul(
                                        ps[:],
                                        w_sb[ct][:, t * C + co * 128:
                                                 t * C + co * 128 + 128],
                                        x_sb[ct][:, y0 + kh:y0 + kh + nrow,
                                                 kw:kw + 62],
                                        start=(ct == 0 and t == 0),
                                        stop=(ct == 1 and t == 8),
                                    )
                            ob = op.tile([128, n], F32, tag="ob")
                            nc.vector.tensor_scalar_add(
                                ob[:], ps[:], b_sb[:, co:co + 1])
                            nc.sync.dma_start(o_d[img, co, :, p0:p0 + n],
                                              ob[:])

            if use_for_i:
                with tc.For_i(0, reps):
                    body()
            else:
                for _ in range(reps):
                    body()

    nc.compile()
    _CACHE[key] = nc
    return nc


def _prep_inputs(x, weight, bias):
    """Host-side shard prep: threshold mask + relayout + bf16 cast."""
    w = np.where(np.abs(weight) < SPARSE_TH, 0.0, weight).astype(np.float32)
    # (cout, cin, kh, kw) -> (cin, kh, kw, cout) -> [2, 128, 9*256]
    wt = np.ascontiguousarray(w.transpose(1, 2, 3, 0)).reshape(
        2, 128, 9 * C).astype(BF_NP)
    b2 = np.ascontiguousarray(bias.astype(np.float32).reshape(2, 128).T)

    n_img = x.shape[0]
    xs = np.ascontiguousarray(
        x.astype(np.float32).reshape(n_img, 64 * 64, C).transpose(0, 2, 1))
    xs = xs.reshape(n_img, 2, 128, 64, 64).astype(BF_NP)

    in_maps = []
    for c in range(N_CORES):
        in_maps.append({
            "xt": np.ascontiguousarray(
                xs[c * IMG_PER_CORE:(c + 1) * IMG_PER_CORE]),
            "wt": wt,
            "bias": b2,
        })
    return in_maps


def _assemble(results):
    outs = np.concatenate([r["out"] for r in results], axis=0)  # (32,2,128,3844)
    outs = outs.reshape(32, C, 62, 62).transpose(0, 2, 3, 1)
    return np.ascontiguousarray(outs)


def kernel(x, weight, bias):
    x = np.asarray(x)
    weight = np.asarray(weight)
    bias = np.asarray(bias)
    nc = _build(reps=1)
    in_maps = _prep_inputs(x, weight, bias)
    res = run_bass_kernel_spmd(nc, in_maps, list(range(N_CORES)))
    return _assemble(res.results)


# revision 5
# speedup vs baseline: 1.1884x; 1.1862x over previous
"""Trainium2 Bass kernel: 3x3 VALID conv (NHWC, 256->256 ch) with weight
thresholding + bias, batch-sharded across 8 NeuronCores (4 images/core).

Device strategy per core:
  - x pre-transposed on host to [cin, H, W] bf16 (2 partition tiles of 128),
    one full-image tile per cin half, double-buffered across images.
  - conv = 9 shifted matmuls per output tile accumulated in PSUM over
    9 taps x 2 cin tiles, bf16 (1 cyc/row, FWL fast weight loads).
  - moving operand is a 3D AP [128, rows, 62] with row stride 64: only the
    62 valid output columns per row are computed (packed output, no
    garbage columns, no padding needed).
  - bias fused into the PSUM->SBUF drain (DVE tensor_scalar_add), fp32 out.
"""

import sys

sys.path.insert(0, "/opt/trn_rl_repo")

import ml_dtypes
import numpy as np

import concourse.bacc as bacc
import concourse.mybir as mybir
import concourse.tile as tile
from concourse.bass_utils import run_bass_kernel_spmd

F32 = mybir.dt.float32
BF16 = mybir.dt.bfloat16
BF_NP = ml_dtypes.bfloat16

N_CORES = 8
IMG_PER_CORE = 4
C = 256
NV = 62 * 62              # 3844 valid output pixels per image
# output blocks: (out_row0, n_out_rows)
BLOCKS = [(8 * b, 8 if b < 7 else 6) for b in range(8)]
SPARSE_TH = 0.01
TAPS = [(kh, kw) for kh in range(3) for kw in range(3)]

_CACHE = {}


def _build(reps: int = 1, use_for_i: bool = False, internal_io: bool = False):
    key = (reps, use_for_i, internal_io)
    if key in _CACHE:
        return _CACHE[key]

    nc = bacc.Bacc("TRN2", target_bir_lowering=False, debug=False,
                   num_devices=N_CORES)

    io_kind = "Internal" if internal_io else None
    x_d = nc.dram_tensor("xt", [IMG_PER_CORE, 2, 128, 64, 64], BF16,
                         kind=io_kind or "ExternalInput")
    w_d = nc.dram_tensor("wt", [2, 128, 9 * C], BF16,
                         kind=io_kind or "ExternalInput")
    b_d = nc.dram_tensor("bias", [128, 2], F32, kind=io_kind or "ExternalInput")
    o_d = nc.dram_tensor("out", [IMG_PER_CORE, 2, 128, NV], F32,
                         kind=io_kind or "ExternalOutput")
    if internal_io:
        di_d = nc.dram_tensor("dummy_in", [1, 4], F32, kind="ExternalInput")
        do_d = nc.dram_tensor("dummy_out", [1, 4], F32, kind="ExternalOutput")

    with tile.TileContext(nc) as tc:
        with tc.tile_pool(name="wp", bufs=1) as wp, \
             tc.tile_pool(name="xp", bufs=3) as xp, \
             tc.tile_pool(name="pp", bufs=8, space="PSUM") as pp, \
             tc.tile_pool(name="op", bufs=8) as op:

            if internal_io:
                dt_sb = wp.tile([1, 4], F32, tag="dummy")
                nc.sync.dma_start(dt_sb[:], di_d[:])
                nc.sync.dma_start(do_d[:], dt_sb[:])

            w_sb = []
            for ct in range(2):
                wt = wp.tile([128, 9 * C], BF16, tag=f"w{ct}")
                nc.sync.dma_start(wt[:], w_d[ct])
                w_sb.append(wt)
            b_sb = wp.tile([128, 2], F32, tag="bias")
            nc.sync.dma_start(b_sb[:], b_d[:])

            def body():
                for img in range(IMG_PER_CORE):
                    x_sb = []
                    for ct in range(2):
                        xt = xp.tile([128, 64, 64], BF16, tag=f"x{ct}")
                        nc.sync.dma_start(xt[:], x_d[img, ct])
                        x_sb.append(xt)
                    for y0, nrow in BLOCKS:
                        n = 62 * nrow
                        p0 = 62 * y0
                        for co in range(2):
                            ps = pp.tile([128, n], F32, tag="ps")
                            for ct in range(2):
                                for t, (kh, kw) in enumerate(TAPS):
                                    nc.tensor.matmul(
                                        ps[:],
                                        w_sb[ct][:, t * C + co * 128:
                                                 t * C + co * 128 + 128],
                                        x_sb[ct][:, y0 + kh:y0 + kh + nrow,
                                                 kw:kw + 62],
                                        start=(ct == 0 and t == 0),
                                        stop=(ct == 1 and t == 8),
                                    )
                            ob = op.tile([128, n], F32, tag="ob")
                            nc.vector.tensor_scalar_add(
                                ob[:], ps[:], b_sb[:, co:co + 1])
                            nc.sync.dma_start(o_d[img, co, :, p0:p0 + n],
                                              ob[:])

            if use_for_i:
                with tc.For_i(0, reps):
                    body()
            else:
                for _ in range(reps):
                    body()

    nc.compile()
    _CACHE[key] = nc
    return nc


def _prep_inputs(x, weight, bias):
    """Host-side shard prep: threshold mask + relayout + bf16 cast."""
    w = np.where(np.abs(weight) < SPARSE_TH, 0.0, weight).astype(np.float32)
    # (cout, cin, kh, kw) -> (cin, kh, kw, cout) -> [2, 128, 9*256]
    wt = np.ascontiguousarray(w.transpose(1, 2, 3, 0)).reshape(
        2, 128, 9 * C).astype(BF_NP)
    b2 = np.ascontiguousarray(bias.astype(np.float32).reshape(2, 128).T)

    n_img = x.shape[0]
    xs = np.ascontiguousarray(
        x.astype(np.float32).reshape(n_img, 64 * 64, C).transpose(0, 2, 1))
    xs = xs.reshape(n_img, 2, 128, 64, 64).astype(BF_NP)

    in_maps = []
    for c in range(N_CORES):
        in_maps.append({
            "xt": np.ascontiguousarray(
                xs[c * IMG_PER_CORE:(c + 1) * IMG_PER_CORE]),
            "wt": wt,
            "bias": b2,
        })
    return in_maps


def _assemble(results):
    outs = np.concatenate([r["out"] for r in results], axis=0)  # (32,2,128,3844)
    outs = outs.reshape(32, C, 62, 62).transpose(0, 2, 3, 1)
    return np.ascontiguousarray(outs)


def kernel(x, weight, bias):
    x = np.asarray(x)
    weight = np.asarray(weight)
    bias = np.asarray(bias)
    nc = _build(reps=1)
    in_maps = _prep_inputs(x, weight, bias)
    res = run_bass_kernel_spmd(nc, in_maps, list(range(N_CORES)))
    return _assemble(res.results)
